# revision 23
# baseline (speedup 1.0000x reference)
"""Multi-head attention (b=4, n=2048, dim=1024, heads=16, hd=64) on 8 TRN2
NeuronCores — head-parallel sharding.

Core i = (batch b = i//2, head-half H = i%2): each core computes Q/K/V for its
8 heads (512 features) over all 2048 tokens, attention for those heads over the
full sequence, and a PARTIAL out-projection over its 512 attn features. The two
cores of a batch return bf16 partials that the host sums (row-parallel
out-proj; ob bias folded into core H=0's partial).

Per-core engine budget: ACT exp(scores) 256 ops x ~1.15us = ~294us is the hard
floor; PE ~272us (QKV 82 + attention 163 + out-proj 27) overlaps under it.
Score matmuls for a head PAIR are row-tiled on the PE array (heads 2p/2p+1 live
at partitions 0-63/64-127 of feature chunk p, so lhsT/rhs base-partitions
auto-derive tile_position (0,0)/(64,0)) and run concurrently. PV keeps the
ones-column trick (M=65) for softmax row sums. Normalization is batched per
q-chunk: sums rows -> DRAM, one reciprocal_approx_fast [8,512], bf16 recips
DMA-broadcast across partitions, one in-SBUF multiply per pair.

Layouts on device (feature-major, partition dim first):
  xT   [128, 8 dc, 2048 t]  x^T, d-chunked
  kT   [128, 4 fc, 2048 t]  K^T local heads (head 2p at [0:64, p], 2p+1 at [64:128, p])
  qT   [128, 4 fc, 2048 t]  Q^T likewise
  v    [128, 16 tt, 8 h, 65] V token-major per local head, col 64 == 1.0
  S^T  psum [128 k, 2 h, 512 q] per k-tile, row-tiled head pair
  attn [128, 4 fc, 2048 t]  UNNORMALIZED P~V (bf16); normalized per qc into
  attn_n [128, 4 fc, 512]   the out-proj input tile
  out  partial [1024 e, 2048 t] bf16 -> host sums core pairs
"""
import sys

sys.path.insert(0, "/opt/trn_rl_repo")

import numpy as np
import ml_dtypes

import concourse.bass as bass
import concourse.tile as tile
from concourse import bacc, mybir
from concourse.bass_utils import run_bass_kernel_spmd

BF16 = mybir.dt.bfloat16
F32 = mybir.dt.float32
EXP = mybir.ActivationFunctionType.Exp
MULT = mybir.AluOpType.mult

D = 1024          # model dim
DC = 8            # d chunks of 128 (contraction for projections)
NT = 2048         # tokens
NHL = 8           # local heads per core
FC = 4            # local feature chunks of 128 (= 512 local features)
QC = 512          # q chunk (psum free)
NQC = 4           # q chunks
NKT = 16          # k tiles of 128
HD = 64           # head dim
N_CORES = 8

_CACHE = {}


def _install_ntff_shim():
    """The agent image's ``antenv`` lacks ``axon_hooks``; recreate the NTFF
    profile glue (same contract as trn_boot's ``_ntff_profile_via_ctypes``)."""
    import types
    import ctypes
    import contextlib

    if "antenv.axon_hooks" in sys.modules:
        return
    so_path = "/opt/axon/libaxon_pjrt.so"
    try:
        lib = ctypes.CDLL(so_path)
        if not hasattr(lib, "axon_start_nrt_profile"):
            return
    except OSError:
        return
    lib.axon_start_nrt_profile.argtypes = [ctypes.POINTER(ctypes.c_int64),
                                           ctypes.c_size_t]
    lib.axon_start_nrt_profile.restype = ctypes.c_int64
    lib.axon_stop_nrt_profile.argtypes = [ctypes.c_char_p]
    lib.axon_stop_nrt_profile.restype = ctypes.c_int64

    @contextlib.contextmanager
    def _hook(output_dir, device_ids):
        import jax
        jax.devices()
        if device_ids:
            ids = (ctypes.c_int64 * len(device_ids))(*device_ids)
            rc = lib.axon_start_nrt_profile(ids, len(device_ids))
        else:
            rc = lib.axon_start_nrt_profile(None, 0)
        if rc != 0:
            raise RuntimeError(f"axon_start_nrt_profile rc={rc}")
        try:
            yield
        finally:
            n = lib.axon_stop_nrt_profile(str(output_dir).encode())
            print(f"ntff profile: {n} file(s) written to {output_dir}",
                  file=sys.stderr)

    mod = types.ModuleType("antenv.axon_hooks")
    _h = [_hook]
    mod.set_axon_ntff_profile_hook = lambda h: _h.__setitem__(0, h)
    mod.get_axon_ntff_profile_hook = lambda: _h[0]
    sys.modules["antenv.axon_hooks"] = mod
    import antenv
    antenv.axon_hooks = mod


def build():
    nc = bacc.Bacc("TRN2", target_bir_lowering=False, debug=False,
                   num_devices=N_CORES)

    xT_d = nc.dram_tensor("xT", [D, NT], BF16, kind="ExternalInput")
    wq_d = nc.dram_tensor("wqT", [D, 512], BF16, kind="ExternalInput")
    wk_d = nc.dram_tensor("wkT", [D, 512], BF16, kind="ExternalInput")
    wv_d = nc.dram_tensor("wvT", [D, 512], BF16, kind="ExternalInput")
    ow_d = nc.dram_tensor("owT", [512, D], BF16, kind="ExternalInput")
    bq_d = nc.dram_tensor("bq", [128, FC], F32, kind="ExternalInput")
    bk_d = nc.dram_tensor("bk", [128, FC], F32, kind="ExternalInput")
    ob_d = nc.dram_tensor("ob", [128, DC], F32, kind="ExternalInput")
    out_d = nc.dram_tensor("outT", [D, NT], BF16, kind="ExternalOutput")

    chunked = lambda t: t.ap().rearrange("(c p) t -> p c t", p=128)

    with tile.TileContext(nc) as tc:
        with tc.tile_pool(name="persist", bufs=1) as persist:
            kT = persist.tile([128, FC, NT], BF16)
            qT = persist.tile([128, FC, NT], BF16)
            v = persist.tile([128, NKT, NHL, HD + 1], BF16)
            attn = persist.tile([128, FC, NT], BF16)
            bq_sb = persist.tile([128, FC], F32)
            bk_sb = persist.tile([128, FC], F32)
            ob_sb = persist.tile([128, DC], F32)
            nc.vector.memset(v, 1.0)   # ones col 64 survives proj epilogues
            warm = persist.tile([128, 1], F32)
            nc.vector.memset(warm, 0.0)

            # PSUM: ps_acc 2x[128,512] (proj/out-proj accum)     = 2 banks
            #       ps_s  2x[128,2,512] (score pair, dbl-buf)    = 4 banks
            #       ps_o  2x[65,512] (PV accum, one per head)    = 2 banks
            with tc.tile_pool(name="w1", bufs=1) as w1, \
                 tc.tile_pool(name="xpool", bufs=1) as xpool, \
                 tc.tile_pool(name="ppool", bufs=16) as ppool, \
                 tc.tile_pool(name="nrm", bufs=2) as nrm, \
                 tc.tile_pool(name="anorm", bufs=2) as anorm, \
                 tc.tile_pool(name="fout", bufs=3) as fout, \
                 tc.tile_pool(name="drpool", bufs=2, space="DRAM") as drpool, \
                 tc.tile_pool(name="ps_acc", bufs=2, space="PSUM") as ps_acc, \
                 tc.tile_pool(name="ps_s", bufs=2, space="PSUM") as ps_s, \
                 tc.tile_pool(name="ps_o", bufs=2, space="PSUM") as ps_o:
                xT = xpool.tile([128, DC, NT], BF16)
                wq = w1.tile([128, DC, 512], BF16)
                wk = w1.tile([128, DC, 512], BF16)
                wv = w1.tile([128, DC, 512], BF16)
                ow = w1.tile([128, FC, D], BF16)
                # readiness order: K0tc0 deps (wk + xT tc0) first, then wq
                # (first scores), wv (V tt0-3 can start on xT tc0), then the
                # remaining xT q-chunks, ow, biases.
                for dc in range(DC):
                    nc.scalar.dma_start(out=wk[:, dc, :],
                                        in_=chunked(wk_d)[:, dc, :])
                    nc.sync.dma_start(out=xT[:, dc, 0:QC],
                                      in_=chunked(xT_d)[:, dc, 0:QC])
                nc.sync.dma_start(out=bk_sb, in_=bk_d.ap())
                nc.sync.dma_start(out=bq_sb, in_=bq_d.ap())
                nc.scalar.dma_start(out=wq, in_=chunked(wq_d))
                nc.gpsimd.dma_start(out=wv, in_=chunked(wv_d))
                for tc in range(1, NQC):
                    for dc in range(DC):
                        nc.sync.dma_start(
                            out=xT[:, dc, tc * QC:(tc + 1) * QC],
                            in_=chunked(xT_d)[:, dc, tc * QC:(tc + 1) * QC])
                nc.gpsimd.dma_start(out=ow, in_=chunked(ow_d))
                nc.sync.dma_start(out=ob_sb, in_=ob_d.ap())
                # pull ACT_TABLE_LOAD off the first real exp's critical path
                nc.scalar.activation(warm, warm, EXP)

                def proj_kq(which, fc, tc):
                    w, b, dst = ((wk, bk_sb, kT) if which == "k"
                                 else (wq, bq_sb, qT))
                    ps = ps_acc.tile([128, QC], F32, tag="ps")
                    for dc in range(DC):
                        nc.tensor.matmul(
                            ps,
                            lhsT=w[:, dc, fc * 128:(fc + 1) * 128],
                            rhs=xT[:, dc, tc * QC:(tc + 1) * QC],
                            start=(dc == 0), stop=(dc == DC - 1))
                    nc.vector.tensor_scalar_add(
                        dst[:, fc, tc * QC:(tc + 1) * QC], ps, b[:, fc:fc + 1])

                def proj_v(tt):
                    ps = ps_acc.tile([128, QC], F32, tag="ps")
                    for dc in range(DC):
                        nc.tensor.matmul(
                            ps,
                            lhsT=xT[:, dc, tt * 128:(tt + 1) * 128],
                            rhs=wv[:, dc, :],
                            start=(dc == 0), stop=(dc == DC - 1))
                    nc.vector.tensor_copy(
                        out=v[:, tt, :, 0:HD],
                        in_=ps.rearrange("p (h d) -> p h d", d=HD))

                # fill queue: (deadline_key, seq, thunk); deadline_key =
                # (window_idx, j) -> must be emitted before that j's PV in
                # that window. Opportunistic early pops are always safe
                # (proj chains depend only on DMAs / earlier-emitted work).
                import heapq
                fill = []
                fill_seq = [0]

                def fill_push(key, thunk):
                    heapq.heappush(fill, (key, fill_seq[0], thunk))
                    fill_seq[0] += 1

                def drain(upto):
                    while fill and fill[0][0] <= upto:
                        heapq.heappop(fill)[2]()

                def pop_one():
                    if fill:
                        heapq.heappop(fill)[2]()

                def attn_pair(p, qc, win):
                    hA, hB = 2 * p, 2 * p + 1
                    qsl = slice(qc * QC, (qc + 1) * QC)
                    po_A = ps_o.tile([HD + 1, QC], F32, tag="po")
                    po_B = ps_o.tile([HD + 1, QC], F32, tag="po")
                    for j in range(NKT):
                        drain((win, j - 4))
                        ksl = slice(j * 128, (j + 1) * 128)
                        ss = ps_s.tile([128, 2, QC], F32, tag="ss")
                        nc.tensor.matmul(
                            ss[:, 0, :], lhsT=kT[0:HD, p, ksl],
                            rhs=qT[0:HD, p, qsl], start=True, stop=True)
                        nc.tensor.matmul(
                            ss[:, 1, :], lhsT=kT[HD:128, p, ksl],
                            rhs=qT[HD:128, p, qsl], start=True, stop=True)
                        pt = ppool.tile([128, 2, QC], BF16, tag="pt")
                        nc.scalar.activation(pt, ss, EXP, scale=0.125)
                        drain((win, j))
                        nc.tensor.matmul(
                            po_A, lhsT=v[:, j, hA, :], rhs=pt[:, 0, :],
                            start=(j == 0), stop=(j == NKT - 1))
                        nc.tensor.matmul(
                            po_B, lhsT=v[:, j, hB, :], rhs=pt[:, 1, :],
                            start=(j == 0), stop=(j == NKT - 1))
                        if j % 5 == 4:
                            pop_one()
                    # epilogues: unnormalized PV -> attn (bf16); the pair's
                    # two PSUM sums rows bounce through DRAM onto partitions
                    # 0-1 (reciprocal_approx_fast needs base-partition 0),
                    # bf16 recips bounce again for the partition broadcast,
                    # then one normalize multiply per pair.
                    sA = nrm.tile([HD + 1, QC], F32, tag="srow")
                    nc.vector.tensor_copy(out=sA[HD:HD + 1, :],
                                          in_=po_A[HD:HD + 1, :])
                    sB = nrm.tile([HD + 1, QC], F32, tag="srow")
                    nc.vector.tensor_copy(out=sB[HD:HD + 1, :],
                                          in_=po_B[HD:HD + 1, :])
                    dsp = drpool.tile([2, QC], F32, tag="dsum", bufs=4)
                    nc.sync.dma_start(out=dsp[0:1, :], in_=sA[HD:HD + 1, :])
                    nc.sync.dma_start(out=dsp[1:2, :], in_=sB[HD:HD + 1, :])
                    rsb = nrm.tile([2, QC], F32, tag="rsb")
                    nc.sync.dma_start(out=rsb, in_=dsp)
                    rc = nrm.tile([2, QC], F32, tag="rc")
                    nc.vector.reciprocal_approx_fast(out=rc, in_=rsb)
                    rcb = nrm.tile([2, QC], BF16, tag="rcb")
                    nc.vector.tensor_copy(out=rcb, in_=rc)
                    drp = drpool.tile([2, QC], BF16, tag="drec", bufs=4)
                    nc.sync.dma_start(out=drp, in_=rcb)
                    bcp = nrm.tile([128, QC], BF16, tag="bc")
                    for half in range(2):
                        row = drp[half:half + 1, :]
                        nc.sync.dma_start(
                            out=bcp[half * HD:(half + 1) * HD, :],
                            in_=bass.AP(tensor=row.tensor, offset=row.offset,
                                        ap=[[0, HD], row.ap[-1]]))
                    nc.vector.tensor_copy(out=attn[0:HD, p, qsl],
                                          in_=po_A[0:HD, :])
                    sh = nrm.tile([HD, QC], BF16, tag="sh")
                    nc.vector.tensor_copy(out=sh, in_=po_B[0:HD, :])
                    nc.gpsimd.dma_start(out=attn[HD:128, p, qsl], in_=sh)
                    an = an_qc[0]
                    nc.vector.tensor_tensor(
                        out=an[:, p, :], in0=attn[:, p, qsl],
                        in1=bcp, op=MULT)

                def out_proj(an, ec, qc):
                    ps = ps_acc.tile([128, QC], F32, tag="ps")
                    for fc in range(FC):
                        nc.tensor.matmul(
                            ps,
                            lhsT=ow[:, fc, ec * 128:(ec + 1) * 128],
                            rhs=an[:, fc, :],
                            start=(fc == 0), stop=(fc == FC - 1))
                    fo = fout.tile([128, QC], BF16, tag="fo")
                    nc.vector.tensor_scalar_add(fo, ps, ob_sb[:, ec:ec + 1])
                    nc.sync.dma_start(
                        out=out_d.ap()[ec * 128:(ec + 1) * 128, qc * QC:(qc + 1) * QC],
                        in_=fo)

                an_qc = [None]

                # ---- emission ----
                # preamble: K fc0 tc0 + Q fc0 tc0 (first scores), V tt0-3
                # (ready on xT tc0 + wv, fills PE while DMAs land).
                proj_kq("k", 0, 0)
                proj_kq("q", 0, 0)
                for tt in range(4):
                    proj_v(tt)

                # deadlines: window w=(qc*4+p); scores j need kT chunk
                # tc=j//4 (deadline (w, 4*tc-4) conservative), qT tc=qc
                # before window; PV j needs v tt=j (deadline (w0, j)).
                for tt in range(4, NKT):
                    fill_push((0, tt - 1), (lambda t: lambda: proj_v(t))(tt))
                for tc in range(1, NQC):
                    fill_push((0, 4 * tc - 4),
                              (lambda t: lambda: proj_kq("k", 0, t))(tc))
                for p in range(1, FC):
                    for tc in range(NQC):
                        fill_push((p, 4 * tc - 4),
                                  (lambda f, t: lambda: proj_kq("k", f, t))(p, tc))
                    fill_push((p, -4),
                              (lambda f: lambda: proj_kq("q", f, 0))(p))
                for qc in range(1, NQC):
                    for p in range(FC):
                        fill_push((qc * 4 + p, -4),
                                  (lambda f, t: lambda: proj_kq("q", f, t))(p, qc))

                for qc in range(NQC):
                    an = anorm.tile([128, FC, QC], BF16, tag="an",
                                    name=f"an{qc}")
                    an_qc[0] = an
                    for p in range(FC):
                        attn_pair(p, qc, qc * 4 + p)
                    if qc < NQC - 1:
                        # spread the 8 out-proj chains over the next qc's
                        # windows so they fill PE slack instead of blocking
                        # the next exp stream
                        for ec in range(DC):
                            fill_push(((qc + 1) * 4 + ec // 4, 4 * (ec % 4)),
                                      (lambda a, e, q: lambda: out_proj(a, e, q))(an, ec, qc))
                    else:
                        for ec in range(DC):
                            out_proj(an, ec, qc)
                drain((99, 99))

    nc.compile()
    return nc


def _prep_in_maps(x, qkv_w, qkv_b, out_w, out_b):
    bf = ml_dtypes.bfloat16
    in_maps = []
    xTs = [np.ascontiguousarray(x[b].T).astype(bf) for b in range(4)]
    halves = []
    for H in range(2):
        fsl = slice(512 * H, 512 * (H + 1))
        wqT = np.ascontiguousarray(qkv_w[0:D][fsl].T).astype(bf)
        wkT = np.ascontiguousarray(qkv_w[D:2 * D][fsl].T).astype(bf)
        wvT = np.ascontiguousarray(qkv_w[2 * D:3 * D][fsl].T).astype(bf)
        owT = np.ascontiguousarray(out_w[:, fsl].T).astype(bf)
        bq = np.ascontiguousarray(
            qkv_b[0:D][fsl].reshape(FC, 128).T).astype(np.float32)
        bk = np.ascontiguousarray(
            qkv_b[D:2 * D][fsl].reshape(FC, 128).T).astype(np.float32)
        ob_eff = out_w[:, fsl] @ qkv_b[2 * D:3 * D][fsl]
        if H == 0:
            ob_eff = ob_eff + out_b
        ob = np.ascontiguousarray(
            ob_eff.reshape(DC, 128).T).astype(np.float32)
        halves.append(dict(wqT=wqT, wkT=wkT, wvT=wvT, owT=owT,
                           bq=bq, bk=bk, ob=ob))
    for i in range(N_CORES):
        b, H = i // 2, i % 2
        in_maps.append(dict(xT=xTs[b], **halves[H]))
    return in_maps


def run(x, qkv_w, qkv_b, out_w, out_b, trace=False):
    if trace:
        _install_ntff_shim()
    if "nc" not in _CACHE:
        _CACHE["nc"] = build()
    nc = _CACHE["nc"]
    in_maps = _prep_in_maps(np.asarray(x, np.float32),
                            np.asarray(qkv_w, np.float32),
                            np.asarray(qkv_b, np.float32),
                            np.asarray(out_w, np.float32),
                            np.asarray(out_b, np.float32))
    res = run_bass_kernel_spmd(nc, in_maps, core_ids=list(range(N_CORES)),
                               trace=trace)
    out = np.empty((4, NT, D), np.float32)
    for b in range(4):
        p0 = res.results[2 * b]["outT"].astype(np.float32)
        p1 = res.results[2 * b + 1]["outT"].astype(np.float32)
        out[b] = (p0 + p1).T
    return out, res


def kernel(**inputs):
    out, _ = run(**inputs)
    return out


# revision 24
# speedup vs baseline: 1.0205x; 1.0205x over previous
"""Multi-head attention (b=4, n=2048, dim=1024, heads=16, hd=64) on 8 TRN2
NeuronCores — head-parallel sharding.

Core i = (batch b = i//2, head-half H = i%2): each core computes Q/K/V for its
8 heads (512 features) over all 2048 tokens, attention for those heads over the
full sequence, and a PARTIAL out-projection over its 512 attn features. The two
cores of a batch return bf16 partials that the host sums (row-parallel
out-proj; ob bias folded into core H=0's partial).

Per-core engine budget: ACT exp(scores) 256 ops x ~1.15us = ~294us is the hard
floor; PE ~272us (QKV 82 + attention 163 + out-proj 27) overlaps under it.
Score matmuls for a head PAIR are row-tiled on the PE array (heads 2p/2p+1 live
at partitions 0-63/64-127 of feature chunk p, so lhsT/rhs base-partitions
auto-derive tile_position (0,0)/(64,0)) and run concurrently. PV keeps the
ones-column trick (M=65) for softmax row sums. Normalization is batched per
q-chunk: sums rows -> DRAM, one reciprocal_approx_fast [8,512], bf16 recips
DMA-broadcast across partitions, one in-SBUF multiply per pair.

Layouts on device (feature-major, partition dim first):
  xT   [128, 8 dc, 2048 t]  x^T, d-chunked
  kT   [128, 4 fc, 2048 t]  K^T local heads (head 2p at [0:64, p], 2p+1 at [64:128, p])
  qT   [128, 4 fc, 2048 t]  Q^T likewise
  v    [128, 16 tt, 8 h, 65] V token-major per local head, col 64 == 1.0
  S^T  psum [128 k, 2 h, 512 q] per k-tile, row-tiled head pair
  attn [128, 4 fc, 2048 t]  UNNORMALIZED P~V (bf16); normalized per qc into
  attn_n [128, 4 fc, 512]   the out-proj input tile
  out  partial [1024 e, 2048 t] bf16 -> host sums core pairs
"""
import sys

sys.path.insert(0, "/opt/trn_rl_repo")

import numpy as np
import ml_dtypes

import concourse.bass as bass
import concourse.tile as tile
from concourse import bacc, mybir
from concourse.bass_utils import run_bass_kernel_spmd

BF16 = mybir.dt.bfloat16
F32 = mybir.dt.float32
EXP = mybir.ActivationFunctionType.Exp
MULT = mybir.AluOpType.mult

D = 1024          # model dim
DC = 8            # d chunks of 128 (contraction for projections)
NT = 2048         # tokens
NHL = 8           # local heads per core
FC = 4            # local feature chunks of 128 (= 512 local features)
QC = 512          # q chunk (psum free)
NQC = 4           # q chunks
NKT = 16          # k tiles of 128
HD = 64           # head dim
N_CORES = 8

_CACHE = {}


def _install_ntff_shim():
    """The agent image's ``antenv`` lacks ``axon_hooks``; recreate the NTFF
    profile glue (same contract as trn_boot's ``_ntff_profile_via_ctypes``)."""
    import types
    import ctypes
    import contextlib

    if "antenv.axon_hooks" in sys.modules:
        return
    so_path = "/opt/axon/libaxon_pjrt.so"
    try:
        lib = ctypes.CDLL(so_path)
        if not hasattr(lib, "axon_start_nrt_profile"):
            return
    except OSError:
        return
    lib.axon_start_nrt_profile.argtypes = [ctypes.POINTER(ctypes.c_int64),
                                           ctypes.c_size_t]
    lib.axon_start_nrt_profile.restype = ctypes.c_int64
    lib.axon_stop_nrt_profile.argtypes = [ctypes.c_char_p]
    lib.axon_stop_nrt_profile.restype = ctypes.c_int64

    @contextlib.contextmanager
    def _hook(output_dir, device_ids):
        import jax
        jax.devices()
        if device_ids:
            ids = (ctypes.c_int64 * len(device_ids))(*device_ids)
            rc = lib.axon_start_nrt_profile(ids, len(device_ids))
        else:
            rc = lib.axon_start_nrt_profile(None, 0)
        if rc != 0:
            raise RuntimeError(f"axon_start_nrt_profile rc={rc}")
        try:
            yield
        finally:
            n = lib.axon_stop_nrt_profile(str(output_dir).encode())
            print(f"ntff profile: {n} file(s) written to {output_dir}",
                  file=sys.stderr)

    mod = types.ModuleType("antenv.axon_hooks")
    _h = [_hook]
    mod.set_axon_ntff_profile_hook = lambda h: _h.__setitem__(0, h)
    mod.get_axon_ntff_profile_hook = lambda: _h[0]
    sys.modules["antenv.axon_hooks"] = mod
    import antenv
    antenv.axon_hooks = mod


def build():
    nc = bacc.Bacc("TRN2", target_bir_lowering=False, debug=False,
                   num_devices=N_CORES)

    xT_d = nc.dram_tensor("xT", [D, NT], BF16, kind="ExternalInput")
    wq_d = nc.dram_tensor("wqT", [D, 512], BF16, kind="ExternalInput")
    wk_d = nc.dram_tensor("wkT", [D, 512], BF16, kind="ExternalInput")
    wv_d = nc.dram_tensor("wvT", [D, 512], BF16, kind="ExternalInput")
    ow_d = nc.dram_tensor("owT", [512, D], BF16, kind="ExternalInput")
    bq_d = nc.dram_tensor("bq", [128, FC], F32, kind="ExternalInput")
    bk_d = nc.dram_tensor("bk", [128, FC], F32, kind="ExternalInput")
    ob_d = nc.dram_tensor("ob", [128, DC], F32, kind="ExternalInput")
    out_d = nc.dram_tensor("outT", [D, NT], BF16, kind="ExternalOutput")

    chunked = lambda t: t.ap().rearrange("(c p) t -> p c t", p=128)

    with tile.TileContext(nc) as tc:
        with tc.tile_pool(name="persist", bufs=1) as persist:
            kT = persist.tile([128, FC, NT], BF16)
            qT = persist.tile([128, FC, NT], BF16)
            v = persist.tile([128, NKT, NHL, HD + 1], BF16)
            attn = persist.tile([128, FC, NT], BF16)
            bq_sb = persist.tile([128, FC], F32)
            bk_sb = persist.tile([128, FC], F32)
            ob_sb = persist.tile([128, DC], F32)
            nc.vector.memset(v, 1.0)   # ones col 64 survives proj epilogues
            warm = persist.tile([128, 1], F32)
            nc.vector.memset(warm, 0.0)

            # PSUM: ps_acc 2x[128,512] (proj/out-proj accum)     = 2 banks
            #       ps_s  2x[128,2,512] (score pair, dbl-buf)    = 4 banks
            #       ps_o  2x[65,512] (PV accum, one per head)    = 2 banks
            with tc.tile_pool(name="w1", bufs=1) as w1, \
                 tc.tile_pool(name="xpool", bufs=1) as xpool, \
                 tc.tile_pool(name="ppool", bufs=16) as ppool, \
                 tc.tile_pool(name="nrm", bufs=2) as nrm, \
                 tc.tile_pool(name="anorm", bufs=2) as anorm, \
                 tc.tile_pool(name="fout", bufs=3) as fout, \
                 tc.tile_pool(name="drpool", bufs=2, space="DRAM") as drpool, \
                 tc.tile_pool(name="ps_acc", bufs=2, space="PSUM") as ps_acc, \
                 tc.tile_pool(name="ps_s", bufs=2, space="PSUM") as ps_s, \
                 tc.tile_pool(name="ps_o", bufs=2, space="PSUM") as ps_o:
                xT = xpool.tile([128, DC, NT], BF16)
                wq = w1.tile([128, DC, 512], BF16)
                wk = w1.tile([128, DC, 512], BF16)
                wv = w1.tile([128, DC, 512], BF16)
                ow = w1.tile([128, FC, D], BF16)
                # readiness order: K0tc0 deps (wk + xT tc0) first, then wq
                # (first scores), wv (V tt0-3 can start on xT tc0), then the
                # remaining xT q-chunks, ow, biases.
                for dc in range(DC):
                    nc.scalar.dma_start(out=wk[:, dc, :],
                                        in_=chunked(wk_d)[:, dc, :])
                    nc.sync.dma_start(out=xT[:, dc, 0:QC],
                                      in_=chunked(xT_d)[:, dc, 0:QC])
                nc.sync.dma_start(out=bk_sb, in_=bk_d.ap())
                nc.sync.dma_start(out=bq_sb, in_=bq_d.ap())
                nc.scalar.dma_start(out=wq, in_=chunked(wq_d))
                nc.gpsimd.dma_start(out=wv, in_=chunked(wv_d))
                for tc in range(1, NQC):
                    for dc in range(DC):
                        nc.sync.dma_start(
                            out=xT[:, dc, tc * QC:(tc + 1) * QC],
                            in_=chunked(xT_d)[:, dc, tc * QC:(tc + 1) * QC])
                nc.gpsimd.dma_start(out=ow, in_=chunked(ow_d))
                nc.sync.dma_start(out=ob_sb, in_=ob_d.ap())
                # pull ACT_TABLE_LOAD off the first real exp's critical path
                nc.scalar.activation(warm, warm, EXP)

                def proj_kq(which, fc, tc):
                    w, b, dst = ((wk, bk_sb, kT) if which == "k"
                                 else (wq, bq_sb, qT))
                    ps = ps_acc.tile([128, QC], F32, tag="ps")
                    for dc in range(DC):
                        nc.tensor.matmul(
                            ps,
                            lhsT=w[:, dc, fc * 128:(fc + 1) * 128],
                            rhs=xT[:, dc, tc * QC:(tc + 1) * QC],
                            start=(dc == 0), stop=(dc == DC - 1))
                    nc.vector.tensor_scalar_add(
                        dst[:, fc, tc * QC:(tc + 1) * QC], ps, b[:, fc:fc + 1])

                def proj_v(tt):
                    ps = ps_acc.tile([128, QC], F32, tag="ps")
                    for dc in range(DC):
                        nc.tensor.matmul(
                            ps,
                            lhsT=xT[:, dc, tt * 128:(tt + 1) * 128],
                            rhs=wv[:, dc, :],
                            start=(dc == 0), stop=(dc == DC - 1))
                    nc.vector.tensor_copy(
                        out=v[:, tt, :, 0:HD],
                        in_=ps.rearrange("p (h d) -> p h d", d=HD))

                # fill queue: (deadline_key, seq, thunk); deadline_key =
                # (window_idx, j) -> must be emitted before that j's PV in
                # that window. Opportunistic early pops are always safe
                # (proj chains depend only on DMAs / earlier-emitted work).
                import heapq
                fill = []
                fill_seq = [0]

                def fill_push(key, thunk):
                    heapq.heappush(fill, (key, fill_seq[0], thunk))
                    fill_seq[0] += 1

                def drain(upto):
                    while fill and fill[0][0] <= upto:
                        heapq.heappop(fill)[2]()

                def pop_one():
                    if fill:
                        heapq.heappop(fill)[2]()

                def attn_pair(p, qc, win):
                    hA, hB = 2 * p, 2 * p + 1
                    qsl = slice(qc * QC, (qc + 1) * QC)
                    po_A = ps_o.tile([HD + 1, QC], F32, tag="po")
                    po_B = ps_o.tile([HD + 1, QC], F32, tag="po")
                    for j in range(NKT):
                        drain((win, j - 4))
                        ksl = slice(j * 128, (j + 1) * 128)
                        ss = ps_s.tile([128, 2, QC], F32, tag="ss")
                        nc.tensor.matmul(
                            ss[:, 0, :], lhsT=kT[0:HD, p, ksl],
                            rhs=qT[0:HD, p, qsl], start=True, stop=True)
                        nc.tensor.matmul(
                            ss[:, 1, :], lhsT=kT[HD:128, p, ksl],
                            rhs=qT[HD:128, p, qsl], start=True, stop=True)
                        pt = ppool.tile([128, 2, QC], BF16, tag="pt")
                        nc.scalar.activation(pt, ss, EXP, scale=0.125)
                        drain((win, j))
                        nc.tensor.matmul(
                            po_A, lhsT=v[:, j, hA, :], rhs=pt[:, 0, :],
                            start=(j == 0), stop=(j == NKT - 1))
                        nc.tensor.matmul(
                            po_B, lhsT=v[:, j, hB, :], rhs=pt[:, 1, :],
                            start=(j == 0), stop=(j == NKT - 1))
                        if j % 5 == 4:
                            pop_one()
                    # epilogues: unnormalized PV -> attn (bf16); the pair's
                    # two PSUM sums rows bounce through DRAM onto partitions
                    # 0-1 (reciprocal_approx_fast needs base-partition 0),
                    # bf16 recips bounce again for the partition broadcast,
                    # then one normalize multiply per pair.
                    s2 = nrm.tile([HD + 1, 2, QC], F32, tag="srow")
                    nc.vector.tensor_copy(out=s2[HD:HD + 1, 0, :],
                                          in_=po_A[HD:HD + 1, :])
                    nc.vector.tensor_copy(out=s2[HD:HD + 1, 1, :],
                                          in_=po_B[HD:HD + 1, :])
                    dsp = drpool.tile([2, QC], F32, tag="dsum", bufs=4)
                    nc.sync.dma_start(out=dsp, in_=s2[HD:HD + 1, :, :])
                    bcs = nrm.tile([128, QC], F32, tag="bcs")
                    for half in range(2):
                        row = dsp[half:half + 1, :]
                        nc.sync.dma_start(
                            out=bcs[half * HD:(half + 1) * HD, :],
                            in_=bass.AP(tensor=row.tensor, offset=row.offset,
                                        ap=[[0, HD], row.ap[-1]]))
                    bcr = nrm.tile([128, QC], F32, tag="bcr")
                    nc.vector.reciprocal_approx_fast(out=bcr, in_=bcs)
                    nc.vector.tensor_copy(out=attn[0:HD, p, qsl],
                                          in_=po_A[0:HD, :])
                    sh = nrm.tile([HD, QC], BF16, tag="sh")
                    nc.vector.tensor_copy(out=sh, in_=po_B[0:HD, :])
                    nc.gpsimd.dma_start(out=attn[HD:128, p, qsl], in_=sh)
                    an = an_qc[0]
                    nc.vector.tensor_tensor(
                        out=an[:, p, :], in0=attn[:, p, qsl],
                        in1=bcr, op=MULT)

                def out_proj(an, ec, qc):
                    ps = ps_acc.tile([128, QC], F32, tag="ps")
                    for fc in range(FC):
                        nc.tensor.matmul(
                            ps,
                            lhsT=ow[:, fc, ec * 128:(ec + 1) * 128],
                            rhs=an[:, fc, :],
                            start=(fc == 0), stop=(fc == FC - 1))
                    fo = fout.tile([128, QC], BF16, tag="fo")
                    nc.vector.tensor_scalar_add(fo, ps, ob_sb[:, ec:ec + 1])
                    nc.sync.dma_start(
                        out=out_d.ap()[ec * 128:(ec + 1) * 128, qc * QC:(qc + 1) * QC],
                        in_=fo)

                an_qc = [None]

                # ---- emission ----
                # preamble: K fc0 tc0 + Q fc0 tc0 (first scores), V tt0-3
                # (ready on xT tc0 + wv, fills PE while DMAs land).
                proj_kq("k", 0, 0)
                proj_kq("q", 0, 0)
                for tt in range(4):
                    proj_v(tt)

                # deadlines: window w=(qc*4+p); scores j need kT chunk
                # tc=j//4 (deadline (w, 4*tc-4) conservative), qT tc=qc
                # before window; PV j needs v tt=j (deadline (w0, j)).
                for tt in range(4, NKT):
                    fill_push((0, tt - 1), (lambda t: lambda: proj_v(t))(tt))
                for tc in range(1, NQC):
                    fill_push((0, 4 * tc - 4),
                              (lambda t: lambda: proj_kq("k", 0, t))(tc))
                for p in range(1, FC):
                    for tc in range(NQC):
                        fill_push((p, 4 * tc - 4),
                                  (lambda f, t: lambda: proj_kq("k", f, t))(p, tc))
                    fill_push((p, -4),
                              (lambda f: lambda: proj_kq("q", f, 0))(p))
                for qc in range(1, NQC):
                    for p in range(FC):
                        fill_push((qc * 4 + p, -4),
                                  (lambda f, t: lambda: proj_kq("q", f, t))(p, qc))

                for qc in range(NQC):
                    an = anorm.tile([128, FC, QC], BF16, tag="an",
                                    name=f"an{qc}")
                    an_qc[0] = an
                    for p in range(FC):
                        attn_pair(p, qc, qc * 4 + p)
                    if qc < NQC - 1:
                        # spread the 8 out-proj chains over the next qc's
                        # windows so they fill PE slack instead of blocking
                        # the next exp stream
                        for ec in range(DC):
                            fill_push(((qc + 1) * 4 + ec // 4, 4 * (ec % 4)),
                                      (lambda a, e, q: lambda: out_proj(a, e, q))(an, ec, qc))
                    else:
                        for ec in range(DC):
                            out_proj(an, ec, qc)
                drain((99, 99))

    nc.compile()
    return nc


def _prep_in_maps(x, qkv_w, qkv_b, out_w, out_b):
    bf = ml_dtypes.bfloat16
    in_maps = []
    xTs = [np.ascontiguousarray(x[b].T).astype(bf) for b in range(4)]
    halves = []
    for H in range(2):
        fsl = slice(512 * H, 512 * (H + 1))
        wqT = np.ascontiguousarray(qkv_w[0:D][fsl].T).astype(bf)
        wkT = np.ascontiguousarray(qkv_w[D:2 * D][fsl].T).astype(bf)
        wvT = np.ascontiguousarray(qkv_w[2 * D:3 * D][fsl].T).astype(bf)
        owT = np.ascontiguousarray(out_w[:, fsl].T).astype(bf)
        bq = np.ascontiguousarray(
            qkv_b[0:D][fsl].reshape(FC, 128).T).astype(np.float32)
        bk = np.ascontiguousarray(
            qkv_b[D:2 * D][fsl].reshape(FC, 128).T).astype(np.float32)
        ob_eff = out_w[:, fsl] @ qkv_b[2 * D:3 * D][fsl]
        if H == 0:
            ob_eff = ob_eff + out_b
        ob = np.ascontiguousarray(
            ob_eff.reshape(DC, 128).T).astype(np.float32)
        halves.append(dict(wqT=wqT, wkT=wkT, wvT=wvT, owT=owT,
                           bq=bq, bk=bk, ob=ob))
    for i in range(N_CORES):
        b, H = i // 2, i % 2
        in_maps.append(dict(xT=xTs[b], **halves[H]))
    return in_maps


def run(x, qkv_w, qkv_b, out_w, out_b, trace=False):
    if trace:
        _install_ntff_shim()
    if "nc" not in _CACHE:
        _CACHE["nc"] = build()
    nc = _CACHE["nc"]
    in_maps = _prep_in_maps(np.asarray(x, np.float32),
                            np.asarray(qkv_w, np.float32),
                            np.asarray(qkv_b, np.float32),
                            np.asarray(out_w, np.float32),
                            np.asarray(out_b, np.float32))
    res = run_bass_kernel_spmd(nc, in_maps, core_ids=list(range(N_CORES)),
                               trace=trace)
    out = np.empty((4, NT, D), np.float32)
    for b in range(4):
        p0 = res.results[2 * b]["outT"].astype(np.float32)
        p1 = res.results[2 * b + 1]["outT"].astype(np.float32)
        out[b] = (p0 + p1).T
    return out, res


def kernel(**inputs):
    out, _ = run(**inputs)
    return out


# revision 29
# speedup vs baseline: 1.0481x; 1.0270x over previous
"""Multi-head attention (b=4, n=2048, dim=1024, heads=16, hd=64) on 8 TRN2
NeuronCores — head-parallel sharding.

Core i = (batch b = i//2, head-half H = i%2): each core computes Q/K/V for its
8 heads (512 features) over all 2048 tokens, attention for those heads over the
full sequence, and a PARTIAL out-projection over its 512 attn features. The two
cores of a batch return bf16 partials that the host sums (row-parallel
out-proj; ob bias folded into core H=0's partial).

Per-core engine budget: ACT exp(scores) 256 ops x ~1.15us = ~294us is the hard
floor; PE ~272us (QKV 82 + attention 163 + out-proj 27) overlaps under it.
Score matmuls for a head PAIR are row-tiled on the PE array (heads 2p/2p+1 live
at partitions 0-63/64-127 of feature chunk p, so lhsT/rhs base-partitions
auto-derive tile_position (0,0)/(64,0)) and run concurrently. PV keeps the
ones-column trick (M=65) for softmax row sums. Normalization is batched per
q-chunk: sums rows -> DRAM, one reciprocal_approx_fast [8,512], bf16 recips
DMA-broadcast across partitions, one in-SBUF multiply per pair.

Layouts on device (feature-major, partition dim first):
  xT   [128, 8 dc, 2048 t]  x^T, d-chunked
  kT   [128, 4 fc, 2048 t]  K^T local heads (head 2p at [0:64, p], 2p+1 at [64:128, p])
  qT   [128, 4 fc, 2048 t]  Q^T likewise
  v    [128, 16 tt, 8 h, 65] V token-major per local head, col 64 == 1.0
  S^T  psum [128 k, 2 h, 512 q] per k-tile, row-tiled head pair
  attn [128, 4 fc, 2048 t]  UNNORMALIZED P~V (bf16); normalized per qc into
  attn_n [128, 4 fc, 512]   the out-proj input tile
  out  partial [1024 e, 2048 t] bf16 -> host sums core pairs
"""
import sys

sys.path.insert(0, "/opt/trn_rl_repo")

import numpy as np
import ml_dtypes

import concourse.bass as bass
import concourse.tile as tile
from concourse import bacc, mybir
from concourse.bass_utils import run_bass_kernel_spmd

BF16 = mybir.dt.bfloat16
F32 = mybir.dt.float32
EXP = mybir.ActivationFunctionType.Exp
MULT = mybir.AluOpType.mult

D = 1024          # model dim
DC = 8            # d chunks of 128 (contraction for projections)
NT = 2048         # tokens
NHL = 8           # local heads per core
FC = 4            # local feature chunks of 128 (= 512 local features)
QC = 512          # q chunk (psum free)
NQC = 4           # q chunks
NKT = 16          # k tiles of 128
HD = 64           # head dim
N_CORES = 8

_CACHE = {}


def _install_ntff_shim():
    """The agent image's ``antenv`` lacks ``axon_hooks``; recreate the NTFF
    profile glue (same contract as trn_boot's ``_ntff_profile_via_ctypes``)."""
    import types
    import ctypes
    import contextlib

    if "antenv.axon_hooks" in sys.modules:
        return
    so_path = "/opt/axon/libaxon_pjrt.so"
    try:
        lib = ctypes.CDLL(so_path)
        if not hasattr(lib, "axon_start_nrt_profile"):
            return
    except OSError:
        return
    lib.axon_start_nrt_profile.argtypes = [ctypes.POINTER(ctypes.c_int64),
                                           ctypes.c_size_t]
    lib.axon_start_nrt_profile.restype = ctypes.c_int64
    lib.axon_stop_nrt_profile.argtypes = [ctypes.c_char_p]
    lib.axon_stop_nrt_profile.restype = ctypes.c_int64

    @contextlib.contextmanager
    def _hook(output_dir, device_ids):
        import jax
        jax.devices()
        if device_ids:
            ids = (ctypes.c_int64 * len(device_ids))(*device_ids)
            rc = lib.axon_start_nrt_profile(ids, len(device_ids))
        else:
            rc = lib.axon_start_nrt_profile(None, 0)
        if rc != 0:
            raise RuntimeError(f"axon_start_nrt_profile rc={rc}")
        try:
            yield
        finally:
            n = lib.axon_stop_nrt_profile(str(output_dir).encode())
            print(f"ntff profile: {n} file(s) written to {output_dir}",
                  file=sys.stderr)

    mod = types.ModuleType("antenv.axon_hooks")
    _h = [_hook]
    mod.set_axon_ntff_profile_hook = lambda h: _h.__setitem__(0, h)
    mod.get_axon_ntff_profile_hook = lambda: _h[0]
    sys.modules["antenv.axon_hooks"] = mod
    import antenv
    antenv.axon_hooks = mod


def build():
    nc = bacc.Bacc("TRN2", target_bir_lowering=False, debug=False,
                   num_devices=N_CORES)

    xT_d = nc.dram_tensor("xT", [D, NT], BF16, kind="ExternalInput")
    wq_d = nc.dram_tensor("wqT", [D, 512], BF16, kind="ExternalInput")
    wk_d = nc.dram_tensor("wkT", [D, 512], BF16, kind="ExternalInput")
    wv_d = nc.dram_tensor("wvT", [D, 512], BF16, kind="ExternalInput")
    ow_d = nc.dram_tensor("owT", [512, D], BF16, kind="ExternalInput")
    bq_d = nc.dram_tensor("bq", [128, FC], F32, kind="ExternalInput")
    bk_d = nc.dram_tensor("bk", [128, FC], F32, kind="ExternalInput")
    ob_d = nc.dram_tensor("ob", [128, DC], F32, kind="ExternalInput")
    out_d = nc.dram_tensor("outT", [D, NT], BF16, kind="ExternalOutput")

    chunked = lambda t: t.ap().rearrange("(c p) t -> p c t", p=128)

    with tile.TileContext(nc) as tc:
        with tc.tile_pool(name="persist", bufs=1) as persist:
            kT = persist.tile([128, FC, NT], BF16)
            qT = persist.tile([128, FC, NT], BF16)
            v = persist.tile([128, NKT, NHL, HD + 1], BF16)
            attn = persist.tile([128, FC, NT], BF16)
            bq_sb = persist.tile([128, FC], F32)
            bk_sb = persist.tile([128, FC], F32)
            ob_sb = persist.tile([128, DC], F32)
            warm = persist.tile([128, 1], F32)
            nc.vector.memset(warm, 0.0)
            # only the ones-column needs init; proj_v fills cols 0-63
            nc.vector.memset(v[:, :, :, HD:HD + 1], 1.0)

            # PSUM: ps_acc 2x[128,512] (proj/out-proj accum)     = 2 banks
            #       ps_s  2x[128,2,512] (score pair, dbl-buf)    = 4 banks
            #       ps_o  2x[65,512] (PV accum, one per head)    = 2 banks
            with tc.tile_pool(name="w1", bufs=1) as w1, \
                 tc.tile_pool(name="xpool", bufs=1) as xpool, \
                 tc.tile_pool(name="ppool", bufs=16) as ppool, \
                 tc.tile_pool(name="nrm", bufs=2) as nrm, \
                 tc.tile_pool(name="anorm", bufs=2) as anorm, \
                 tc.tile_pool(name="fout", bufs=3) as fout, \
                 tc.tile_pool(name="drpool", bufs=2, space="DRAM") as drpool, \
                 tc.tile_pool(name="ps_acc", bufs=2, space="PSUM") as ps_acc, \
                 tc.tile_pool(name="ps_s", bufs=2, space="PSUM") as ps_s, \
                 tc.tile_pool(name="ps_o", bufs=2, space="PSUM") as ps_o:
                xT = xpool.tile([128, DC, NT], BF16)
                wq = w1.tile([128, DC, 512], BF16)
                wk = w1.tile([128, DC, 512], BF16)
                wv = w1.tile([128, DC, 512], BF16)
                ow = w1.tile([128, FC, D], BF16)
                # readiness order: K0tc0 deps (wk + xT tc0) first, then wq
                # (first scores), wv (V tt0-3 can start on xT tc0), then the
                # remaining xT q-chunks, ow, biases.
                for dc in range(DC):
                    nc.scalar.dma_start(out=wk[:, dc, :],
                                        in_=chunked(wk_d)[:, dc, :])
                    nc.sync.dma_start(out=xT[:, dc, 0:QC],
                                      in_=chunked(xT_d)[:, dc, 0:QC])
                nc.sync.dma_start(out=bk_sb, in_=bk_d.ap())
                nc.sync.dma_start(out=bq_sb, in_=bq_d.ap())
                nc.scalar.dma_start(out=wq, in_=chunked(wq_d))
                nc.gpsimd.dma_start(out=wv, in_=chunked(wv_d))
                for tc in range(1, NQC):
                    for dc in range(DC):
                        nc.sync.dma_start(
                            out=xT[:, dc, tc * QC:(tc + 1) * QC],
                            in_=chunked(xT_d)[:, dc, tc * QC:(tc + 1) * QC])
                nc.gpsimd.dma_start(out=ow, in_=chunked(ow_d))
                nc.sync.dma_start(out=ob_sb, in_=ob_d.ap())
                # pull ACT_TABLE_LOAD off the first real exp's critical path
                nc.scalar.activation(warm, warm, EXP)

                def proj_kq(which, fc, tc):
                    w, b, dst = ((wk, bk_sb, kT) if which == "k"
                                 else (wq, bq_sb, qT))
                    ps = ps_acc.tile([128, QC], F32, tag="ps")
                    for dc in range(DC):
                        nc.tensor.matmul(
                            ps,
                            lhsT=w[:, dc, fc * 128:(fc + 1) * 128],
                            rhs=xT[:, dc, tc * QC:(tc + 1) * QC],
                            start=(dc == 0), stop=(dc == DC - 1))
                    nc.vector.tensor_scalar_add(
                        dst[:, fc, tc * QC:(tc + 1) * QC], ps, b[:, fc:fc + 1])

                def proj_v(tt):
                    ps = ps_acc.tile([128, QC], F32, tag="ps")
                    for dc in range(DC):
                        nc.tensor.matmul(
                            ps,
                            lhsT=xT[:, dc, tt * 128:(tt + 1) * 128],
                            rhs=wv[:, dc, :],
                            start=(dc == 0), stop=(dc == DC - 1))
                    nc.vector.tensor_copy(
                        out=v[:, tt, :, 0:HD],
                        in_=ps.rearrange("p (h d) -> p h d", d=HD))

                # fill queue: (deadline_key, seq, thunk); deadline_key =
                # (window_idx, j) -> must be emitted before that j's PV in
                # that window. Opportunistic early pops are always safe
                # (proj chains depend only on DMAs / earlier-emitted work).
                import heapq
                fill = []
                fill_seq = [0]

                def fill_push(key, thunk):
                    heapq.heappush(fill, (key, fill_seq[0], thunk))
                    fill_seq[0] += 1

                def drain(upto):
                    while fill and fill[0][0] <= upto:
                        heapq.heappop(fill)[2]()

                def pop_one():
                    if fill:
                        heapq.heappop(fill)[2]()

                def attn_pair(p, qc, win, prev_tail):
                    hA, hB = 2 * p, 2 * p + 1
                    qsl = slice(qc * QC, (qc + 1) * QC)
                    po_A = ps_o.tile([HD + 1, QC], F32, tag="po")
                    po_B = ps_o.tile([HD + 1, QC], F32, tag="po")
                    pts = {}

                    def pv(jj, last):
                        nc.tensor.matmul(
                            po_A, lhsT=v[:, jj, hA, :], rhs=pts[jj][:, 0, :],
                            start=(jj == 0), stop=last)
                        nc.tensor.matmul(
                            po_B, lhsT=v[:, jj, hB, :], rhs=pts.pop(jj)[:, 1, :],
                            start=(jj == 0), stop=last)

                    for j in range(NKT):
                        drain((win, j - 4))
                        ksl = slice(j * 128, (j + 1) * 128)
                        ss = ps_s.tile([128, 2, QC], F32, tag="ss")
                        nc.tensor.matmul(
                            ss[:, 0, :], lhsT=kT[0:HD, p, ksl],
                            rhs=qT[0:HD, p, qsl], start=True, stop=True)
                        nc.tensor.matmul(
                            ss[:, 1, :], lhsT=kT[HD:128, p, ksl],
                            rhs=qT[HD:128, p, qsl], start=True, stop=True)
                        pt = ppool.tile([128, 2, QC], BF16, tag="pt")
                        nc.scalar.activation(pt, ss, EXP, scale=0.125)
                        pts[j] = pt
                        # previous window's deferred tail goes here so its
                        # last PVs sit BEHIND our first scores in the PE
                        # FIFO -- the next exp never waits on them
                        if j == 1 and prev_tail is not None:
                            prev_tail()
                        drain((win, j))
                        if j >= 2:
                            pv(j - 2, False)
                        if j % 5 == 4:
                            pop_one()
                    an_cur = an_qc[0]
                    return lambda: _pair_tail(p, qsl, po_A, po_B, pv, an_cur)

                def _pair_tail(p, qsl, po_A, po_B, pv, an):
                    pv(NKT - 2, False)
                    pv(NKT - 1, True)
                    # epilogues: unnormalized PV -> attn (bf16); the pair's
                    # two PSUM sums rows bounce through DRAM onto partitions
                    # 0-1 (reciprocal_approx_fast needs base-partition 0),
                    # bf16 recips bounce again for the partition broadcast,
                    # then one normalize multiply per pair.
                    s2 = nrm.tile([HD + 1, 2, QC], F32, tag="srow")
                    nc.vector.tensor_copy(out=s2[HD:HD + 1, 0, :],
                                          in_=po_A[HD:HD + 1, :])
                    nc.vector.tensor_copy(out=s2[HD:HD + 1, 1, :],
                                          in_=po_B[HD:HD + 1, :])
                    dsp = drpool.tile([2, QC], F32, tag="dsum", bufs=4)
                    nc.sync.dma_start(out=dsp, in_=s2[HD:HD + 1, :, :])
                    bcs = nrm.tile([128, QC], F32, tag="bcs")
                    for half in range(2):
                        row = dsp[half:half + 1, :]
                        nc.sync.dma_start(
                            out=bcs[half * HD:(half + 1) * HD, :],
                            in_=bass.AP(tensor=row.tensor, offset=row.offset,
                                        ap=[[0, HD], row.ap[-1]]))
                    bcr = nrm.tile([128, QC], F32, tag="bcr")
                    nc.vector.reciprocal_approx_fast(out=bcr, in_=bcs)
                    nc.vector.tensor_copy(out=attn[0:HD, p, qsl],
                                          in_=po_A[0:HD, :])
                    sh = nrm.tile([HD, QC], BF16, tag="sh")
                    nc.vector.tensor_copy(out=sh, in_=po_B[0:HD, :])
                    nc.gpsimd.dma_start(out=attn[HD:128, p, qsl], in_=sh)
                    nc.vector.tensor_tensor(
                        out=an[:, p, :], in0=attn[:, p, qsl],
                        in1=bcr, op=MULT)

                def out_proj(an, ec, qc):
                    ps = ps_acc.tile([128, QC], F32, tag="ps")
                    for fc in range(FC):
                        nc.tensor.matmul(
                            ps,
                            lhsT=ow[:, fc, ec * 128:(ec + 1) * 128],
                            rhs=an[:, fc, :],
                            start=(fc == 0), stop=(fc == FC - 1))
                    fo = fout.tile([128, QC], BF16, tag="fo")
                    nc.vector.tensor_scalar_add(fo, ps, ob_sb[:, ec:ec + 1])
                    nc.sync.dma_start(
                        out=out_d.ap()[ec * 128:(ec + 1) * 128, qc * QC:(qc + 1) * QC],
                        in_=fo)

                an_qc = [None]

                # ---- emission ----
                # preamble: K fc0 tc0 + Q fc0 tc0 (first scores), V tt0-3
                # (ready on xT tc0 + wv, fills PE while DMAs land).
                proj_kq("k", 0, 0)
                proj_kq("q", 0, 0)
                for tt in range(4):
                    proj_v(tt)

                # deadlines: window w=(qc*4+p); scores j need kT chunk
                # tc=j//4 (deadline (w, 4*tc-4) conservative), qT tc=qc
                # before window; PV j needs v tt=j (deadline (w0, j)).
                for tt in range(4, NKT):
                    fill_push((0, tt - 1), (lambda t: lambda: proj_v(t))(tt))
                for tc in range(1, NQC):
                    fill_push((0, 4 * tc - 4),
                              (lambda t: lambda: proj_kq("k", 0, t))(tc))
                for p in range(1, FC):
                    for tc in range(NQC):
                        fill_push((p, 4 * tc - 4),
                                  (lambda f, t: lambda: proj_kq("k", f, t))(p, tc))
                    fill_push((p, -4),
                              (lambda f: lambda: proj_kq("q", f, 0))(p))
                for qc in range(1, NQC):
                    for p in range(FC):
                        fill_push((qc * 4 + p, -4),
                                  (lambda f, t: lambda: proj_kq("q", f, t))(p, qc))

                tail = None
                for qc in range(NQC):
                    an = anorm.tile([128, FC, QC], BF16, tag="an",
                                    name=f"an{qc}")
                    an_qc[0] = an
                    for p in range(FC):
                        tail = attn_pair(p, qc, qc * 4 + p, tail)
                    if qc < NQC - 1:
                        # spread the 8 out-proj chains over the next qc's
                        # windows (keys j>=2: the deferred pair tail lands
                        # at j==1 and the out-proj MMs must sit behind the
                        # an-completing multiply in the PE FIFO)
                        for ec in range(DC):
                            fill_push(((qc + 1) * 4 + ec // 4, 2 + 3 * (ec % 4)),
                                      (lambda a, e, q: lambda: out_proj(a, e, q))(an, ec, qc))
                tail()
                for ec in range(DC):
                    out_proj(an_qc[0], ec, NQC - 1)
                drain((99, 99))

    nc.compile()
    return nc


def _prep_in_maps(x, qkv_w, qkv_b, out_w, out_b):
    bf = ml_dtypes.bfloat16
    in_maps = []
    xTs = [np.ascontiguousarray(x[b].T).astype(bf) for b in range(4)]
    halves = []
    for H in range(2):
        fsl = slice(512 * H, 512 * (H + 1))
        wqT = np.ascontiguousarray(qkv_w[0:D][fsl].T).astype(bf)
        wkT = np.ascontiguousarray(qkv_w[D:2 * D][fsl].T).astype(bf)
        wvT = np.ascontiguousarray(qkv_w[2 * D:3 * D][fsl].T).astype(bf)
        owT = np.ascontiguousarray(out_w[:, fsl].T).astype(bf)
        bq = np.ascontiguousarray(
            qkv_b[0:D][fsl].reshape(FC, 128).T).astype(np.float32)
        bk = np.ascontiguousarray(
            qkv_b[D:2 * D][fsl].reshape(FC, 128).T).astype(np.float32)
        ob_eff = out_w[:, fsl] @ qkv_b[2 * D:3 * D][fsl]
        if H == 0:
            ob_eff = ob_eff + out_b
        ob = np.ascontiguousarray(
            ob_eff.reshape(DC, 128).T).astype(np.float32)
        halves.append(dict(wqT=wqT, wkT=wkT, wvT=wvT, owT=owT,
                           bq=bq, bk=bk, ob=ob))
    for i in range(N_CORES):
        b, H = i // 2, i % 2
        in_maps.append(dict(xT=xTs[b], **halves[H]))
    return in_maps


def run(x, qkv_w, qkv_b, out_w, out_b, trace=False):
    if trace:
        _install_ntff_shim()
    if "nc" not in _CACHE:
        _CACHE["nc"] = build()
    nc = _CACHE["nc"]
    in_maps = _prep_in_maps(np.asarray(x, np.float32),
                            np.asarray(qkv_w, np.float32),
                            np.asarray(qkv_b, np.float32),
                            np.asarray(out_w, np.float32),
                            np.asarray(out_b, np.float32))
    res = run_bass_kernel_spmd(nc, in_maps, core_ids=list(range(N_CORES)),
                               trace=trace)
    out = np.empty((4, NT, D), np.float32)
    for b in range(4):
        p0 = res.results[2 * b]["outT"].astype(np.float32)
        p1 = res.results[2 * b + 1]["outT"].astype(np.float32)
        out[b] = (p0 + p1).T
    return out, res


def kernel(**inputs):
    out, _ = run(**inputs)
    return out


# revision 32
# speedup vs baseline: 1.0560x; 1.0075x over previous
"""Multi-head attention (b=4, n=2048, dim=1024, heads=16, hd=64) on 8 TRN2
NeuronCores — head-parallel sharding.

Core i = (batch b = i//2, head-half H = i%2): each core computes Q/K/V for its
8 heads (512 features) over all 2048 tokens, attention for those heads over the
full sequence, and a PARTIAL out-projection over its 512 attn features. The two
cores of a batch return bf16 partials that the host sums (row-parallel
out-proj; ob bias folded into core H=0's partial).

Per-core engine budget: ACT exp(scores) 256 ops x ~1.15us = ~294us is the hard
floor; PE ~272us (QKV 82 + attention 163 + out-proj 27) overlaps under it.
Score matmuls for a head PAIR are row-tiled on the PE array (heads 2p/2p+1 live
at partitions 0-63/64-127 of feature chunk p, so lhsT/rhs base-partitions
auto-derive tile_position (0,0)/(64,0)) and run concurrently. PV keeps the
ones-column trick (M=65) for softmax row sums. Normalization is batched per
q-chunk: sums rows -> DRAM, one reciprocal_approx_fast [8,512], bf16 recips
DMA-broadcast across partitions, one in-SBUF multiply per pair.

Layouts on device (feature-major, partition dim first):
  xT   [128, 8 dc, 2048 t]  x^T, d-chunked
  kT   [128, 4 fc, 2048 t]  K^T local heads (head 2p at [0:64, p], 2p+1 at [64:128, p])
  qT   [128, 4 fc, 2048 t]  Q^T likewise
  v    [128, 16 tt, 8 h, 65] V token-major per local head, col 64 == 1.0
  S^T  psum [128 k, 2 h, 512 q] per k-tile, row-tiled head pair
  attn [128, 4 fc, 2048 t]  UNNORMALIZED P~V (bf16); normalized per qc into
  attn_n [128, 4 fc, 512]   the out-proj input tile
  out  partial [1024 e, 2048 t] bf16 -> host sums core pairs
"""
import sys

sys.path.insert(0, "/opt/trn_rl_repo")

import numpy as np
import ml_dtypes

import concourse.bass as bass
import concourse.tile as tile
from concourse import bacc, mybir
from concourse.bass_utils import run_bass_kernel_spmd

BF16 = mybir.dt.bfloat16
F32 = mybir.dt.float32
EXP = mybir.ActivationFunctionType.Exp
MULT = mybir.AluOpType.mult

D = 1024          # model dim
DC = 8            # d chunks of 128 (contraction for projections)
NT = 2048         # tokens
NHL = 8           # local heads per core
FC = 4            # local feature chunks of 128 (= 512 local features)
QC = 512          # q chunk (psum free)
NQC = 4           # q chunks
NKT = 16          # k tiles of 128
HD = 64           # head dim
N_CORES = 8

_CACHE = {}


def _install_ntff_shim():
    """The agent image's ``antenv`` lacks ``axon_hooks``; recreate the NTFF
    profile glue (same contract as trn_boot's ``_ntff_profile_via_ctypes``)."""
    import types
    import ctypes
    import contextlib

    if "antenv.axon_hooks" in sys.modules:
        return
    so_path = "/opt/axon/libaxon_pjrt.so"
    try:
        lib = ctypes.CDLL(so_path)
        if not hasattr(lib, "axon_start_nrt_profile"):
            return
    except OSError:
        return
    lib.axon_start_nrt_profile.argtypes = [ctypes.POINTER(ctypes.c_int64),
                                           ctypes.c_size_t]
    lib.axon_start_nrt_profile.restype = ctypes.c_int64
    lib.axon_stop_nrt_profile.argtypes = [ctypes.c_char_p]
    lib.axon_stop_nrt_profile.restype = ctypes.c_int64

    @contextlib.contextmanager
    def _hook(output_dir, device_ids):
        import jax
        jax.devices()
        if device_ids:
            ids = (ctypes.c_int64 * len(device_ids))(*device_ids)
            rc = lib.axon_start_nrt_profile(ids, len(device_ids))
        else:
            rc = lib.axon_start_nrt_profile(None, 0)
        if rc != 0:
            raise RuntimeError(f"axon_start_nrt_profile rc={rc}")
        try:
            yield
        finally:
            n = lib.axon_stop_nrt_profile(str(output_dir).encode())
            print(f"ntff profile: {n} file(s) written to {output_dir}",
                  file=sys.stderr)

    mod = types.ModuleType("antenv.axon_hooks")
    _h = [_hook]
    mod.set_axon_ntff_profile_hook = lambda h: _h.__setitem__(0, h)
    mod.get_axon_ntff_profile_hook = lambda: _h[0]
    sys.modules["antenv.axon_hooks"] = mod
    import antenv
    antenv.axon_hooks = mod


def build():
    nc = bacc.Bacc("TRN2", target_bir_lowering=False, debug=False,
                   num_devices=N_CORES)

    xT_d = nc.dram_tensor("xT", [D, NT], BF16, kind="ExternalInput")
    wq_d = nc.dram_tensor("wqT", [D, 512], BF16, kind="ExternalInput")
    wk_d = nc.dram_tensor("wkT", [D, 512], BF16, kind="ExternalInput")
    wv_d = nc.dram_tensor("wvT", [D, 512], BF16, kind="ExternalInput")
    ow_d = nc.dram_tensor("owT", [512, D], BF16, kind="ExternalInput")
    bq_d = nc.dram_tensor("bq", [128, FC], F32, kind="ExternalInput")
    bk_d = nc.dram_tensor("bk", [128, FC], F32, kind="ExternalInput")
    ob_d = nc.dram_tensor("ob", [128, DC], F32, kind="ExternalInput")
    out_d = nc.dram_tensor("outT", [D, NT], BF16, kind="ExternalOutput")

    chunked = lambda t: t.ap().rearrange("(c p) t -> p c t", p=128)

    with tile.TileContext(nc) as tc:
        with tc.tile_pool(name="persist", bufs=1) as persist:
            kT = persist.tile([128, FC, NT], BF16)
            qT = persist.tile([128, FC, NT], BF16)
            v = persist.tile([128, NKT, NHL, HD + 1], BF16)
            attn = persist.tile([128, FC, NT], BF16)
            bq_sb = persist.tile([128, FC], F32)
            bk_sb = persist.tile([128, FC], F32)
            ob_sb = persist.tile([128, DC], F32)
            warm = persist.tile([128, 1], F32)
            nc.vector.memset(warm, 0.0)
            # only the ones-column needs init; proj_v fills cols 0-63
            nc.vector.memset(v[:, :, :, HD:HD + 1], 1.0)

            # PSUM: ps_acc 2x[128,512] (proj/out-proj accum)     = 2 banks
            #       ps_s  2x[128,2,512] (score pair, dbl-buf)    = 4 banks
            #       ps_o  2x[65,512] (PV accum, one per head)    = 2 banks
            with tc.tile_pool(name="w1", bufs=1) as w1, \
                 tc.tile_pool(name="xpool", bufs=1) as xpool, \
                 tc.tile_pool(name="ppool", bufs=16) as ppool, \
                 tc.tile_pool(name="nrm", bufs=2) as nrm, \
                 tc.tile_pool(name="anorm", bufs=2) as anorm, \
                 tc.tile_pool(name="fout", bufs=3) as fout, \
                 tc.tile_pool(name="drpool", bufs=2, space="DRAM") as drpool, \
                 tc.tile_pool(name="ps_acc", bufs=2, space="PSUM") as ps_acc, \
                 tc.tile_pool(name="ps_s", bufs=2, space="PSUM") as ps_s, \
                 tc.tile_pool(name="ps_o", bufs=2, space="PSUM") as ps_o:
                xT = xpool.tile([128, DC, NT], BF16)
                wq = w1.tile([128, DC, 512], BF16)
                wk = w1.tile([128, DC, 512], BF16)
                wv = w1.tile([128, DC, 512], BF16)
                ow = w1.tile([128, FC, D], BF16)
                # readiness order: K0tc0 deps (wk + xT tc0) first, then wq
                # (first scores), wv (V tt0-3 can start on xT tc0), then the
                # remaining xT q-chunks, ow, biases.
                for dc in range(DC):
                    nc.scalar.dma_start(out=wk[:, dc, :],
                                        in_=chunked(wk_d)[:, dc, :])
                    nc.sync.dma_start(out=xT[:, dc, 0:QC],
                                      in_=chunked(xT_d)[:, dc, 0:QC])
                nc.sync.dma_start(out=bk_sb, in_=bk_d.ap())
                nc.sync.dma_start(out=bq_sb, in_=bq_d.ap())
                nc.scalar.dma_start(out=wq, in_=chunked(wq_d))
                nc.gpsimd.dma_start(out=wv, in_=chunked(wv_d))
                for tc in range(1, NQC):
                    for dc in range(DC):
                        nc.sync.dma_start(
                            out=xT[:, dc, tc * QC:(tc + 1) * QC],
                            in_=chunked(xT_d)[:, dc, tc * QC:(tc + 1) * QC])
                nc.gpsimd.dma_start(out=ow, in_=chunked(ow_d))
                nc.sync.dma_start(out=ob_sb, in_=ob_d.ap())
                # pull ACT_TABLE_LOAD off the first real exp's critical path
                nc.scalar.activation(warm, warm, EXP)

                def chain_kq(which, fc, tc):
                    # split into two 4-dc halves so fill pops insert at most
                    # ~1us of PE work between score batches; the psum
                    # accumulation group stays open across the gap
                    w, b, dst = ((wk, bk_sb, kT) if which == "k"
                                 else (wq, bq_sb, qT))
                    st = {}

                    def half(lo):
                        def emit():
                            if lo == 0:
                                st["ps"] = ps_acc.tile([128, QC], F32,
                                                       tag="ps", name="ps")
                            ps = st["ps"]
                            for dc in range(lo, lo + DC // 2):
                                nc.tensor.matmul(
                                    ps,
                                    lhsT=w[:, dc, fc * 128:(fc + 1) * 128],
                                    rhs=xT[:, dc, tc * QC:(tc + 1) * QC],
                                    start=(dc == 0), stop=(dc == DC - 1))
                            if lo:
                                nc.vector.tensor_scalar_add(
                                    dst[:, fc, tc * QC:(tc + 1) * QC], ps,
                                    b[:, fc:fc + 1])
                        return emit
                    return half(0), half(DC // 2)

                def proj_kq(which, fc, tc):
                    h1, h2 = chain_kq(which, fc, tc)
                    h1(); h2()

                def chain_v(tt):
                    st = {}

                    def half(lo):
                        def emit():
                            if lo == 0:
                                st["ps"] = ps_acc.tile([128, QC], F32,
                                                       tag="ps", name="ps")
                            ps = st["ps"]
                            for dc in range(lo, lo + DC // 2):
                                nc.tensor.matmul(
                                    ps,
                                    lhsT=xT[:, dc, tt * 128:(tt + 1) * 128],
                                    rhs=wv[:, dc, :],
                                    start=(dc == 0), stop=(dc == DC - 1))
                            if lo:
                                nc.vector.tensor_copy(
                                    out=v[:, tt, :, 0:HD],
                                    in_=ps.rearrange("p (h d) -> p h d", d=HD))
                        return emit
                    return half(0), half(DC // 2)

                def proj_v(tt):
                    h1, h2 = chain_v(tt)
                    h1(); h2()

                # fill queue: (deadline_key, seq, thunk); deadline_key =
                # (window_idx, j) -> must be emitted before that j's PV in
                # that window. Opportunistic early pops are always safe
                # (proj chains depend only on DMAs / earlier-emitted work).
                import heapq
                fill = []
                fill_seq = [0]

                def fill_push(key, thunk):
                    heapq.heappush(fill, (key, fill_seq[0], thunk))
                    fill_seq[0] += 1

                def drain(upto):
                    while fill and fill[0][0] <= upto:
                        heapq.heappop(fill)[2]()

                def pop_one():
                    if fill:
                        heapq.heappop(fill)[2]()

                def attn_pair(p, qc, win, prev_tail):
                    hA, hB = 2 * p, 2 * p + 1
                    qsl = slice(qc * QC, (qc + 1) * QC)
                    po_A = ps_o.tile([HD + 1, QC], F32, tag="po")
                    po_B = ps_o.tile([HD + 1, QC], F32, tag="po")
                    pts = {}

                    def pv(jj, last):
                        nc.tensor.matmul(
                            po_A, lhsT=v[:, jj, hA, :], rhs=pts[jj][:, 0, :],
                            start=(jj == 0), stop=last)
                        nc.tensor.matmul(
                            po_B, lhsT=v[:, jj, hB, :], rhs=pts.pop(jj)[:, 1, :],
                            start=(jj == 0), stop=last)

                    for j in range(NKT):
                        drain((win, j - 4))
                        ksl = slice(j * 128, (j + 1) * 128)
                        ss = ps_s.tile([128, 2, QC], F32, tag="ss")
                        nc.tensor.matmul(
                            ss[:, 0, :], lhsT=kT[0:HD, p, ksl],
                            rhs=qT[0:HD, p, qsl], start=True, stop=True)
                        nc.tensor.matmul(
                            ss[:, 1, :], lhsT=kT[HD:128, p, ksl],
                            rhs=qT[HD:128, p, qsl], start=True, stop=True)
                        pt = ppool.tile([128, 2, QC], BF16, tag="pt")
                        nc.scalar.activation(pt, ss, EXP, scale=0.125)
                        pts[j] = pt
                        # previous window's deferred tail goes here so its
                        # last PVs sit BEHIND our first scores in the PE
                        # FIFO -- the next exp never waits on them
                        if j == 1 and prev_tail is not None:
                            prev_tail()
                        drain((win, j))
                        if j >= 2:
                            pv(j - 2, False)
                        if j % 3 == 2:
                            pop_one()
                    an_cur = an_qc[0]
                    return lambda: _pair_tail(p, qsl, po_A, po_B, pv, an_cur)

                def _pair_tail(p, qsl, po_A, po_B, pv, an):
                    pv(NKT - 2, False)
                    pv(NKT - 1, True)
                    # epilogues: unnormalized PV -> attn (bf16); the pair's
                    # two PSUM sums rows bounce through DRAM onto partitions
                    # 0-1 (reciprocal_approx_fast needs base-partition 0),
                    # bf16 recips bounce again for the partition broadcast,
                    # then one normalize multiply per pair.
                    s2 = nrm.tile([HD + 1, 2, QC], F32, tag="srow")
                    nc.vector.tensor_copy(out=s2[HD:HD + 1, 0, :],
                                          in_=po_A[HD:HD + 1, :])
                    nc.vector.tensor_copy(out=s2[HD:HD + 1, 1, :],
                                          in_=po_B[HD:HD + 1, :])
                    dsp = drpool.tile([2, QC], F32, tag="dsum", bufs=4)
                    nc.sync.dma_start(out=dsp, in_=s2[HD:HD + 1, :, :])
                    bcs = nrm.tile([128, QC], F32, tag="bcs")
                    for half in range(2):
                        row = dsp[half:half + 1, :]
                        nc.sync.dma_start(
                            out=bcs[half * HD:(half + 1) * HD, :],
                            in_=bass.AP(tensor=row.tensor, offset=row.offset,
                                        ap=[[0, HD], row.ap[-1]]))
                    bcr = nrm.tile([128, QC], F32, tag="bcr")
                    nc.vector.reciprocal_approx_fast(out=bcr, in_=bcs)
                    nc.vector.tensor_copy(out=attn[0:HD, p, qsl],
                                          in_=po_A[0:HD, :])
                    sh = nrm.tile([HD, QC], BF16, tag="sh")
                    nc.vector.tensor_copy(out=sh, in_=po_B[0:HD, :])
                    nc.gpsimd.dma_start(out=attn[HD:128, p, qsl], in_=sh)
                    nc.vector.tensor_tensor(
                        out=an[:, p, :], in0=attn[:, p, qsl],
                        in1=bcr, op=MULT)

                def out_proj(an, ec, qc):
                    ps = ps_acc.tile([128, QC], F32, tag="ps")
                    for fc in range(FC):
                        nc.tensor.matmul(
                            ps,
                            lhsT=ow[:, fc, ec * 128:(ec + 1) * 128],
                            rhs=an[:, fc, :],
                            start=(fc == 0), stop=(fc == FC - 1))
                    fo = fout.tile([128, QC], BF16, tag="fo")
                    nc.vector.tensor_scalar_add(fo, ps, ob_sb[:, ec:ec + 1])
                    nc.sync.dma_start(
                        out=out_d.ap()[ec * 128:(ec + 1) * 128, qc * QC:(qc + 1) * QC],
                        in_=fo)

                an_qc = [None]

                # ---- emission ----
                # preamble: K fc0 tc0 + Q fc0 tc0 (first scores), V tt0-3
                # (ready on xT tc0 + wv, fills PE while DMAs land).
                proj_kq("k", 0, 0)
                proj_kq("q", 0, 0)
                for tt in range(4):
                    proj_v(tt)

                # deadlines: window w=(qc*4+p); scores j need kT chunk
                # tc=j//4 (deadline (w, 4*tc-4) conservative), qT tc=qc
                # before window; PV j needs v tt=j (deadline (w0, j)).
                def push_halves(key, halves):
                    for h in halves:
                        fill_push(key, h)

                for tt in range(4, NKT):
                    push_halves((0, tt - 1), chain_v(tt))
                for tc in range(1, NQC):
                    push_halves((0, 4 * tc - 4), chain_kq("k", 0, tc))
                for p in range(1, FC):
                    for tc in range(NQC):
                        push_halves((p, 4 * tc - 4), chain_kq("k", p, tc))
                    push_halves((p, -4), chain_kq("q", p, 0))
                for qc in range(1, NQC):
                    for p in range(FC):
                        push_halves((qc * 4 + p, -4), chain_kq("q", p, qc))

                tail = None
                for qc in range(NQC):
                    an = anorm.tile([128, FC, QC], BF16, tag="an",
                                    name=f"an{qc}")
                    an_qc[0] = an
                    for p in range(FC):
                        tail = attn_pair(p, qc, qc * 4 + p, tail)
                    if qc < NQC - 1:
                        # spread the 8 out-proj chains over the next qc's
                        # windows (keys j>=2: the deferred pair tail lands
                        # at j==1 and the out-proj MMs must sit behind the
                        # an-completing multiply in the PE FIFO)
                        for ec in range(DC):
                            fill_push(((qc + 1) * 4 + ec // 4, 2 + 3 * (ec % 4)),
                                      (lambda a, e, q: lambda: out_proj(a, e, q))(an, ec, qc))
                tail()
                for ec in range(DC):
                    out_proj(an_qc[0], ec, NQC - 1)
                drain((99, 99))

    nc.compile()
    return nc


def _prep_in_maps(x, qkv_w, qkv_b, out_w, out_b):
    bf = ml_dtypes.bfloat16
    in_maps = []
    xTs = [np.ascontiguousarray(x[b].T).astype(bf) for b in range(4)]
    halves = []
    for H in range(2):
        fsl = slice(512 * H, 512 * (H + 1))
        wqT = np.ascontiguousarray(qkv_w[0:D][fsl].T).astype(bf)
        wkT = np.ascontiguousarray(qkv_w[D:2 * D][fsl].T).astype(bf)
        wvT = np.ascontiguousarray(qkv_w[2 * D:3 * D][fsl].T).astype(bf)
        owT = np.ascontiguousarray(out_w[:, fsl].T).astype(bf)
        bq = np.ascontiguousarray(
            qkv_b[0:D][fsl].reshape(FC, 128).T).astype(np.float32)
        bk = np.ascontiguousarray(
            qkv_b[D:2 * D][fsl].reshape(FC, 128).T).astype(np.float32)
        ob_eff = out_w[:, fsl] @ qkv_b[2 * D:3 * D][fsl]
        if H == 0:
            ob_eff = ob_eff + out_b
        ob = np.ascontiguousarray(
            ob_eff.reshape(DC, 128).T).astype(np.float32)
        halves.append(dict(wqT=wqT, wkT=wkT, wvT=wvT, owT=owT,
                           bq=bq, bk=bk, ob=ob))
    for i in range(N_CORES):
        b, H = i // 2, i % 2
        in_maps.append(dict(xT=xTs[b], **halves[H]))
    return in_maps


def run(x, qkv_w, qkv_b, out_w, out_b, trace=False):
    if trace:
        _install_ntff_shim()
    if "nc" not in _CACHE:
        _CACHE["nc"] = build()
    nc = _CACHE["nc"]
    in_maps = _prep_in_maps(np.asarray(x, np.float32),
                            np.asarray(qkv_w, np.float32),
                            np.asarray(qkv_b, np.float32),
                            np.asarray(out_w, np.float32),
                            np.asarray(out_b, np.float32))
    res = run_bass_kernel_spmd(nc, in_maps, core_ids=list(range(N_CORES)),
                               trace=trace)
    out = np.empty((4, NT, D), np.float32)
    for b in range(4):
        p0 = res.results[2 * b]["outT"].astype(np.float32)
        p1 = res.results[2 * b + 1]["outT"].astype(np.float32)
        out[b] = (p0 + p1).T
    return out, res


def kernel(**inputs):
    out, _ = run(**inputs)
    return out
